# revision 1
# baseline (speedup 1.0000x reference)
"""Trainium2 Bass kernel for nn_MultiHeadAttention (B=4, T=2048, D=1024, H=16).

Sharding: tensor-parallel over heads - 2 heads per core on 8 cores.
Per core: QKV projections (fp32r matmuls) for its 2 heads, causal flash-style
attention with scores kept transposed (s^T[t_k, t_q]); the causal mask is
applied on the tensor engine by accumulating a triangular -1e30 constant into
the scores PSUM; the softmax denominator comes from a ones-column appended to
V. An AllToAll reshards from head-split to token-split, then the output
projection runs on each core's 1024-token slice.

kernel(**inputs) takes the full unsharded inputs and returns the full output.
"""
import sys
sys.path.insert(0, "/opt/trn_rl_repo")
import numpy as np

B, T, D, H = 4, 2048, 1024, 16
DK = D // H           # 64
NCORES = 8
HPC = H // NCORES     # 2 heads per core
TB = B * T            # 8192 tokens
TS = TB // NCORES     # 1024-token slice per core in output projection
NKT = D // 128        # 8 contraction k-tiles
NEG = -1.0e30


def build_nc(repeat=1, with_collective=True, parts="all", mdt="f32r"):
    import concourse.bacc as bacc
    import concourse.tile as tile
    import concourse.mybir as mybir

    f32 = mybir.dt.float32
    f32r = mybir.dt.bfloat16 if mdt == "bf16" else mybir.dt.float32r
    AF = mybir.ActivationFunctionType

    nc = bacc.Bacc("TRN2", target_bir_lowering=False, debug=False,
                   num_devices=NCORES)

    # ---- I/O (all matmul-feeding tensors are f32r; bytes == fp32) ----
    xt = nc.dram_tensor("xt", [D, TB], f32r, kind="ExternalInput")
    wqkv = nc.dram_tensor("wqkv", [128, NKT, 3 * 128], f32r, kind="ExternalInput")
    bqkv = nc.dram_tensor("bqkv", [128, 3], f32, kind="ExternalInput")
    trif = nc.dram_tensor("trif", [128, 2, 512], f32r, kind="ExternalInput")
    idr = nc.dram_tensor("idr", [128, 128], f32r, kind="ExternalInput")
    wot = nc.dram_tensor("wot", [128, NKT, D], f32r, kind="ExternalInput")
    bov = nc.dram_tensor("bov", [1, D], f32r, kind="ExternalInput")
    onesd = nc.dram_tensor("onesd", [128, 16], f32r, kind="ExternalInput")
    y = nc.dram_tensor("y", [TS, D], f32, kind="ExternalOutput")

    # collective buffers: plain internal DRAM tensors (pool tiles crash A2A)
    agin = nc.dram_tensor("agin", [NCORES, 128, TS], f32r)
    agout = nc.dram_tensor("agout", [NCORES, 128, TS], f32r)

    xt_r = xt.ap().rearrange("(k p) t -> p k t", p=128)

    with tile.TileContext(nc) as tc:
        with tc.tile_pool(name="const", bufs=1) as cpool, \
             tc.tile_pool(name="xin", bufs=2) as xpool, \
             tc.tile_pool(name="qkv", bufs=2) as qkvpool, \
             tc.tile_pool(name="vt", bufs=2) as vtpool, \
             tc.tile_pool(name="st", bufs=6) as stpool, \
             tc.tile_pool(name="att", bufs=2) as attpool, \
             tc.tile_pool(name="ysb", bufs=2) as ypool, \
             tc.tile_pool(name="small", bufs=4) as smpool, \
             tc.tile_pool(name="ps", bufs=2, space="PSUM") as ps, \
             tc.tile_pool(name="ps1", bufs=2, space="PSUM") as ps1, \
             tc.tile_pool(name="psat", bufs=2, space="PSUM") as psat:

            # ---- constants ----
            wqkv_sb = cpool.tile([128, NKT, 3 * 128], f32r, tag="wqkv")
            nc.sync.dma_start(wqkv_sb[:], wqkv.ap())
            bqkv_sb = cpool.tile([128, 3], f32, tag="bqkv")
            nc.sync.dma_start(bqkv_sb[:], bqkv.ap())
            tri_sb = cpool.tile([128, 2, 512], f32r, tag="tri")
            nc.sync.dma_start(tri_sb[:], trif.ap())
            idr_sb = cpool.tile([128, 128], f32r, tag="idr")
            nc.sync.dma_start(idr_sb[:], idr.ap())
            idn = cpool.tile([128, 128], f32 if mdt == "f32r" else f32r, tag="idn")
            nc.sync.dma_start(idn[:], idr.ap() if mdt == "bf16" else idr.ap().bitcast(f32))
            wot_sb = cpool.tile([128, NKT, D], f32r, tag="wot")
            nc.sync.dma_start(wot_sb[:], wot.ap())
            bov_sb = cpool.tile([1, D], f32r, tag="bov")
            nc.sync.dma_start(bov_sb[:], bov.ap())
            ones16 = cpool.tile([128, 16], f32r, tag="ones16")
            nc.sync.dma_start(ones16[:], onesd.ap())
            ones_sb = cpool.tile([1, 128], f32r, tag="ones")
            nc.sync.dma_start(
                ones_sb[:],
                onesd.ap()[0:8, :].rearrange("p a -> (p a)").rearrange(
                    "(a b) -> a b", a=1))

            for rep in range(repeat):
                attn_body(nc, mybir, AF, f32, f32r, xt_r, agin,
                          wqkv_sb, bqkv_sb, tri_sb, idr_sb, idn, ones_sb, ones16,
                          xpool, qkvpool, vtpool, stpool, attpool, smpool,
                          ps, ps1, psat, parts, mdt)
                if parts != "all":
                    continue

                if with_collective:
                    nc.gpsimd.collective_compute(
                        "AllToAll", mybir.AluOpType.bypass,
                        replica_groups=[list(range(NCORES))],
                        ins=[agin.ap().opt()], outs=[agout.ap().opt()])
                    src = agout.ap()
                else:
                    src = agin.ap()  # timing variant: wrong data, same shapes

                # ---- output projection for our 1024-token slice ----
                for tch in range(2):
                    yh = xpool.tile([128, NKT, 512], f32r, tag="xin")
                    nc.sync.dma_start(
                        yh[:],
                        src.rearrange("k p t -> p k t")[:, :, tch * 512:(tch + 1) * 512])
                    for tt in range(4):
                        y_sb = ypool.tile([128, D], f32, tag="y")
                        for eb in range(2):
                            pt = ps1.tile([128, 512], f32, tag="p1")
                            for kk in range(NKT):
                                nc.tensor.matmul(
                                    pt[:],
                                    yh[:, kk, tt * 128:tt * 128 + 128],
                                    wot_sb[:, kk, eb * 512:eb * 512 + 512],
                                    start=(kk == 0), stop=False)
                            nc.tensor.matmul(pt[:], ones_sb[0:1, 0:128],
                                             bov_sb[0:1, eb * 512:eb * 512 + 512],
                                             start=False, stop=True)
                            nc.vector.tensor_copy(y_sb[:, eb * 512:eb * 512 + 512],
                                                  pt[:])
                        nc.sync.dma_start(
                            y.ap()[tch * 512 + tt * 128: tch * 512 + tt * 128 + 128, :],
                            y_sb[:])
    nc.compile()
    return nc


def attn_body(nc, mybir, AF, f32, f32r, xt_r, agin,
              wqkv_sb, bqkv_sb, tri_sb, idr_sb, idn, ones_sb, ones16,
              xpool, qkvpool, vtpool, stpool, attpool, smpool,
              ps, ps1, psat, parts, mdt="f32r"):
    for b in range(B):
        t0b = b * T
        qT = qkvpool.tile([128, T], f32r, tag="qT")
        kT = qkvpool.tile([128, T], f32r, tag="kT")
        v_b = qkvpool.tile([128, 16, 130], f32r, tag="v")
        attT = attpool.tile([128, T], f32r, tag="attT")

        # ---- P1: q^T, k^T, v^T projections + v transpose ----
        for blk in range(4):
            t0 = t0b + blk * 512
            xin = xpool.tile([128, NKT, 512], f32r, tag="xin")
            nc.sync.dma_start(xin[:], xt_r[:, :, t0:t0 + 512])
            vT = vtpool.tile([128, 512], f32 if mdt == 'f32r' else f32r, tag="vT")
            for pi, dest in ((0, qT), (1, kT), (2, vT)):
                pt = ps1.tile([128, 512], f32, tag="p1")
                for kk in range(NKT):
                    nc.tensor.matmul(pt[:],
                                     wqkv_sb[:, kk, pi * 128:pi * 128 + 128],
                                     xin[:, kk, :],
                                     start=(kk == 0), stop=(kk == NKT - 1))
                if pi == 0:
                    nc.vector.tensor_scalar_add(dest[:, blk * 512:blk * 512 + 512],
                                                pt[:], bqkv_sb[:, 0:1])
                elif pi == 1:
                    nc.vector.tensor_scalar_add(dest[:, blk * 512:blk * 512 + 512],
                                                pt[:], bqkv_sb[:, 1:2])
                else:
                    nc.vector.tensor_scalar_add(dest[:, 0:512],
                                                pt[:], bqkv_sb[:, 2:3])
            # transpose v^T[128, 512] -> v tiles [t, 130]
            pt = ps1.tile([128, 512], f32 if mdt == "f32r" else f32r, tag="p1")
            for tt in range(4):
                nc.tensor.transpose(pt[:, tt * 128:tt * 128 + 128],
                                    vT[:, tt * 128:tt * 128 + 128], idn[:])
            for tt in range(4):
                j = blk * 4 + tt
                nc.vector.tensor_copy(v_b[:, j, 0:64], pt[:, tt * 128:tt * 128 + 64])
                nc.vector.tensor_copy(v_b[:, j, 65:129],
                                      pt[:, tt * 128 + 64:tt * 128 + 128])
        nc.vector.tensor_copy(v_b[:, :, 64:65],
                              ones16[:].rearrange("p (a c) -> p a c", c=1))
        nc.vector.tensor_copy(v_b[:, :, 129:130],
                              ones16[:].rearrange("p (a c) -> p a c", c=1))

        # ---- P2 ----
        if parts == "p1":
            nc.sync.dma_start(agin.ap()[2 * b, :, :], qT[:, 0:TS])
            nc.sync.dma_start(agin.ap()[2 * b + 1, :, :], kT[:, 0:TS])
            nc.sync.dma_start(agin.ap()[2 * b, :, 0:130], v_b[:, 0, :])
            continue

        for qblk in range(4):
            njt = 4 * qblk + 4
            q0 = qblk * 512
            js = list(range(njt))
            groups = [js[i:i + 2] for i in range(0, njt, 2)]
            at = [psat.tile([65, 512], f32, tag="at", name=f"at{qblk}_{hh}")
                  for hh in range(HPC)]
            for g in groups:
                c0s = {}
                for j in g:
                    c0 = max(0, 128 * j - q0)
                    c0s[j] = (c0, min(c0, 256))
                pt = []
                stg = []
                for h in range(HPC):
                    qh = qT[64 * h:64 * h + 64, :]
                    kh = kT[64 * h:64 * h + 64, :]
                    pth = ps.tile([128, 1024], f32, tag="sc", name=f"sc{h}_{g[0]}")
                    sth = stpool.tile([128, 2, 512], f32r, tag="st",
                                      name=f"st{h}_{g[0]}")
                    pt.append(pth)
                    stg.append(sth)
                    for jj, j in enumerate(g):
                        c0, c0a = c0s[j]
                        diag = 128 * j >= q0
                        nc.tensor.matmul(pth[:, jj * 512 + c0a: jj * 512 + 512],
                                         kh[:, 128 * j:128 * j + 128],
                                         qh[:, q0 + c0a: q0 + 512],
                                         start=True, stop=not diag)
                        if diag:
                            # accumulate -1e30 upper-left triangle (causal mask)
                            kk4 = (128 * j - q0) // 128
                            var = 1 if kk4 == 3 else 0
                            w = 512 - c0a
                            nc.tensor.matmul(
                                pth[:, jj * 512 + c0a: jj * 512 + 512],
                                idr_sb[:], tri_sb[:, var, 0:w],
                                start=False, stop=True)
                # exp: always one instruction per (h, group)
                for h in range(HPC):
                    nc.scalar.activation(
                        stg[h][:, 0:len(g), :].rearrange("p a b -> p (a b)"),
                        pt[h][:, 0:len(g) * 512],
                        AF.Exp, scale=0.125)
                if parts == "p12se":
                    for h in range(HPC):
                        nc.vector.tensor_copy(attT[:, q0:q0 + 512],
                                              stg[h][:, 0, :])
                    continue
                # att accumulation
                for h in range(HPC):
                    for jj, j in enumerate(g):
                        c0, c0a = c0s[j]
                        nc.tensor.matmul(at[h][:, c0a:512],
                                         v_b[:, j, 65 * h:65 * h + 65],
                                         stg[h][:, jj, c0a:512],
                                         start=(j == 0), stop=(j == njt - 1))
            if parts == "p12se":
                continue
            # normalize: reciprocal of denom row, PE K=1 broadcast matmul
            for h in range(HPC):
                recip = smpool.tile([1, 512], f32r, tag="recip")
                with nc.allow_low_precision(reason="fp32r recip feeds matmul"):
                    nc.vector.reciprocal(recip[:], at[h][64:65, :])
                bc = ps1.tile([128, 512], f32, tag="p1", name=f"bc{qblk}_{h}")
                nc.tensor.matmul(bc[0:64, :], ones_sb[0:1, 0:64], recip[:],
                                 start=True, stop=True)
                bc_sb = smpool.tile([64, 512], f32, tag="bcsb")
                nc.vector.tensor_copy(bc_sb[:], bc[0:64, :])
                nc.vector.tensor_mul(attT[64 * h:64 * h + 64, q0:q0 + 512],
                                     at[h][0:64, :], bc_sb[:])
        # ship attT to A2A input
        nc.sync.dma_start(agin.ap()[2 * b, :, :], attT[:, 0:TS])
        nc.sync.dma_start(agin.ap()[2 * b + 1, :, :], attT[:, TS:T])


# ------------------------------------------------------------------
# Host-side wrapper
# ------------------------------------------------------------------
_CACHE = {}


def _prep_inputs(x, wq, bq, wk, bk, wv, bv, wo, bo, mdt="f32r"):
    if mdt == "bf16":
        import ml_dtypes
        cast = lambda a: np.asarray(a, ml_dtypes.bfloat16)
    else:
        cast = lambda a: np.asarray(a, np.float32)
    xt = np.ascontiguousarray(x.reshape(TB, D).T)          # [D, TB]

    def lhsT_pack(W):   # W [128, D] -> [128p, NKT, 128m]
        return np.ascontiguousarray(W.T.reshape(NKT, 128, 128).transpose(1, 0, 2))

    # triangular -1e30 mask constants
    trif = np.zeros((128, 2, 512), np.float32)
    cols = np.arange(512)
    for p in range(128):
        trif[p, 0, :] = np.where(cols < p, NEG, 0.0)
        trif[p, 1, :] = np.where(cols < p + 128, NEG, 0.0)

    wott = np.ascontiguousarray(wo.T.reshape(NKT, 128, D).transpose(1, 0, 2))
    bov = bo.reshape(1, D).astype(np.float32)
    idr = np.eye(128, dtype=np.float32)

    in_maps = []
    for c in range(NCORES):
        h0, h1 = HPC * c, HPC * c + 1
        Wq = np.concatenate([wq[h0], wq[h1]], axis=0)      # [128, D]
        Wk = np.concatenate([wk[h0], wk[h1]], axis=0)
        Wv = np.concatenate([wv[h0], wv[h1]], axis=0)
        wqkvp = np.concatenate([lhsT_pack(Wq), lhsT_pack(Wk), lhsT_pack(Wv)],
                               axis=2)                     # [128, NKT, 384]
        bqkvp = np.stack([np.concatenate([bq[h0], bq[h1]]),
                          np.concatenate([bk[h0], bk[h1]]),
                          np.concatenate([bv[h0], bv[h1]])], axis=1)  # [128,3]
        in_maps.append({
            "xt": cast(xt),
            "wqkv": cast(np.ascontiguousarray(wqkvp, np.float32)),
            "bqkv": np.ascontiguousarray(bqkvp, np.float32),
            "trif": cast(trif),
            "idr": cast(idr),
            "wot": cast(wott),
            "bov": cast(bov),
            "onesd": cast(np.ones((128, 16), np.float32)),
        })
    return in_maps


MDT = "f32r"   # matmul dtype: "f32r" (~2.3e-4 rel err) or "bf16" (~10-20% faster, ~3.5e-3)


def kernel(x, wq, bq, wk, bk, wv, bv, wo, bo):
    from concourse import bass_utils
    x, wq, bq, wk, bk, wv, bv, wo, bo = (
        np.asarray(a, np.float32) for a in (x, wq, bq, wk, bk, wv, bv, wo, bo))
    if "nc" not in _CACHE:
        _CACHE["nc"] = build_nc(mdt=MDT)
    nc = _CACHE["nc"]
    in_maps = _prep_inputs(x, wq, bq, wk, bk, wv, bv, wo, bo, mdt=MDT)
    res = bass_utils.run_bass_kernel_spmd(nc, in_maps, core_ids=list(range(NCORES)))
    ys = [res.results[c]["y"] for c in range(NCORES)]
    return np.concatenate(ys, axis=0).reshape(B, T, D)



# revision 15
# speedup vs baseline: 2.2746x; 2.2746x over previous
"""Trainium2 Bass kernel for nn_MultiHeadAttention (B=4, T=2048, D=1024, H=16).

Sharding: tensor-parallel over heads - 2 heads per core on 8 cores.
Per core: QKV projections for its 2 heads, causal attention with scores kept
transposed (s^T[t_k, t_q]); the causal mask is a single 128-wide triangular
-1e30 constant accumulated into the diagonal score block on the tensor
engine; the softmax denominator comes from a ones-column appended to V.
Normalization is deferred: unnormalized attention + per-(head,token)
denominators ship through per-batch AllToAlls (head-split -> 256-token-slice
split), and the receiving core folds 1/Z in via a selector matmul before the
output projection. The output projection for batch b runs overlapped with
attention for batch b+1.

kernel(**inputs) takes the full unsharded inputs and returns the full output.
"""
import sys
sys.path.insert(0, "/opt/trn_rl_repo")
import numpy as np

B, T, D, H = 4, 2048, 1024, 16
DK = D // H           # 64
NCORES = 8
HPC = H // NCORES     # 2 heads per core
TB = B * T            # 8192 tokens
TS = TB // NCORES     # 1024 tokens of output per core (4 batches x 256)
NKT = D // 128        # 8 contraction k-tiles
TSL = T // NCORES     # 256-token A2A slice
NEG = -1.0e30
LAG = 3               # AV trails scores by LAG j-tiles


def build_nc(repeat=1, with_collective=True, parts="all", mdt="bf16"):
    import concourse.bacc as bacc
    import concourse.tile as tile
    import concourse.mybir as mybir

    f32 = mybir.dt.float32
    f32r = mybir.dt.bfloat16 if mdt == "bf16" else mybir.dt.float32r
    AF = mybir.ActivationFunctionType

    nc = bacc.Bacc("TRN2", target_bir_lowering=False, debug=False,
                   num_devices=NCORES)

    xt = nc.dram_tensor("xt", [D, TB], f32r, kind="ExternalInput")
    wqkv = nc.dram_tensor("wqkv", [128, NKT, 3 * 128], f32r, kind="ExternalInput")
    bqkv = nc.dram_tensor("bqkv", [128, 3], f32, kind="ExternalInput")
    trif = nc.dram_tensor("trif", [128, 128], f32r, kind="ExternalInput")
    idr = nc.dram_tensor("idr", [128, 128], f32r, kind="ExternalInput")
    self_sel = nc.dram_tensor("sel", [16, NKT * 128], f32r, kind="ExternalInput")
    wot = nc.dram_tensor("wot", [128, NKT, D], f32r, kind="ExternalInput")
    bov = nc.dram_tensor("bov", [1, D], f32r, kind="ExternalInput")
    onesd = nc.dram_tensor("onesd", [128, 16], f32r, kind="ExternalInput")
    y = nc.dram_tensor("y", [TS, D], f32, kind="ExternalOutput")

    # collective buffers: per-batch A2A payload = att rows 0:128 + denom 128:130
    agin = [nc.dram_tensor(f"agin{b}", [NCORES, 130, TSL], f32r)
            for b in range(B)]
    agout = [nc.dram_tensor(f"agout{b}", [NCORES, 130, TSL], f32r)
             for b in range(B)]

    xt_r = xt.ap().rearrange("(k p) t -> p k t", p=128)

    with tile.TileContext(nc) as tc:
        with tc.tile_pool(name="const", bufs=1) as cpool, \
             tc.tile_pool(name="xin", bufs=6) as xpool, \
             tc.tile_pool(name="qkv", bufs=2) as qkvpool, \
             tc.tile_pool(name="vt", bufs=2) as vtpool, \
             tc.tile_pool(name="st", bufs=4) as stpool, \
             tc.tile_pool(name="att", bufs=2) as attpool, \
             tc.tile_pool(name="yh", bufs=2) as yhpool, \
             tc.tile_pool(name="ysb", bufs=2) as ypool, \
             tc.tile_pool(name="small", bufs=4) as smpool, \
             tc.tile_pool(name="ps", bufs=2, space="PSUM") as ps, \
             tc.tile_pool(name="ps1", bufs=2, space="PSUM") as ps1, \
             tc.tile_pool(name="psat", bufs=2, space="PSUM") as psat:

            # ---- constants ----
            wqkv_sb = cpool.tile([128, NKT, 3 * 128], f32r, tag="wqkv")
            nc.sync.dma_start(wqkv_sb[:], wqkv.ap())
            bqkv_sb = cpool.tile([128, 3], f32, tag="bqkv")
            nc.sync.dma_start(bqkv_sb[:], bqkv.ap())
            tri_sb = cpool.tile([128, 128], f32r, tag="tri")
            nc.sync.dma_start(tri_sb[:], trif.ap())
            idr_sb = cpool.tile([128, 128], f32r, tag="idr")
            nc.sync.dma_start(idr_sb[:], idr.ap())
            idn = cpool.tile([128, 128], f32 if mdt == "f32r" else f32r, tag="idn")
            nc.sync.dma_start(idn[:], idr.ap() if mdt == "bf16" else idr.ap().bitcast(f32))
            sel_sb = cpool.tile([16, NKT * 128], f32r, tag="sel")
            nc.sync.dma_start(sel_sb[:], self_sel.ap())
            wot_sb = cpool.tile([128, NKT, D], f32r, tag="wot")
            nc.sync.dma_start(wot_sb[:], wot.ap())
            bov_sb = cpool.tile([1, D], f32r, tag="bov")
            nc.sync.dma_start(bov_sb[:], bov.ap())
            ones16 = cpool.tile([128, 16], f32r, tag="ones16")
            nc.sync.dma_start(ones16[:], onesd.ap())
            ones_sb = cpool.tile([1, 128], f32r, tag="ones")
            nc.sync.dma_start(
                ones_sb[:],
                onesd.ap()[0:8, :].rearrange("p a -> (p a)").rearrange(
                    "(a b) -> a b", a=1))

            def prefetch_x(b):
                tiles = []
                for blk in range(4):
                    t0 = b * T + blk * 512
                    xin = xpool.tile([128, NKT, 512], f32r, tag="xin",
                                     name=f"xin{b}_{blk}")
                    nc.sync.dma_start(xin[:], xt_r[:, :, t0:t0 + 512])
                    tiles.append(xin)
                return tiles

            def emit_p1(b, xtiles):
                """QKV projections + v transpose for batch b."""
                qT = qkvpool.tile([128, T], f32r, tag="qT")
                kT = qkvpool.tile([128, T], f32r, tag="kT")
                v_b = qkvpool.tile([128, 16, 130], f32r, tag="v")
                for blk in range(4):
                    xin = xtiles[blk]
                    vT = vtpool.tile([128, 512], f32 if mdt == "f32r" else f32r,
                                     tag="vT")
                    for pi, dest in ((0, qT), (1, kT), (2, vT)):
                        pt = ps1.tile([128, 512], f32, tag="p1")
                        for kk in range(NKT):
                            nc.tensor.matmul(pt[:],
                                             wqkv_sb[:, kk, pi * 128:pi * 128 + 128],
                                             xin[:, kk, :],
                                             start=(kk == 0), stop=(kk == NKT - 1))
                        if pi == 0:
                            nc.vector.tensor_scalar_add(
                                dest[:, blk * 512:blk * 512 + 512],
                                pt[:], bqkv_sb[:, 0:1])
                        elif pi == 1:
                            nc.vector.tensor_scalar_add(
                                dest[:, blk * 512:blk * 512 + 512],
                                pt[:], bqkv_sb[:, 1:2])
                        else:
                            nc.vector.tensor_scalar_add(dest[:, 0:512],
                                                        pt[:], bqkv_sb[:, 2:3])
                    # transpose v^T[128, 512] -> v tiles [t, 130]
                    pt = ps1.tile([128, 512], f32 if mdt == "f32r" else f32r,
                                  tag="p1")
                    for tt in range(4):
                        nc.tensor.transpose(pt[:, tt * 128:tt * 128 + 128],
                                            vT[:, tt * 128:tt * 128 + 128], idn[:])
                    for tt in range(4):
                        j = blk * 4 + tt
                        nc.vector.tensor_copy(v_b[:, j, 0:64],
                                              pt[:, tt * 128:tt * 128 + 64])
                        nc.vector.tensor_copy(v_b[:, j, 65:129],
                                              pt[:, tt * 128 + 64:tt * 128 + 128])
                nc.vector.tensor_copy(v_b[:, :, 64:65],
                                      ones16[:].rearrange("p (a c) -> p a c", c=1))
                nc.vector.tensor_copy(v_b[:, :, 129:130],
                                      ones16[:].rearrange("p (a c) -> p a c", c=1))
                return qT, kT, v_b

            def emit_p2(b, qT, kT, v_b, mid_emit=None):
                """Causal attention for batch b -> attT [128, T] + denB [2, T]
                (unnormalized; row 64 of each at tile is the denominator).
                One [128, 1024] score PSUM tile per j holds both heads, so
                exp is a single ACT instruction per j."""
                attT = attpool.tile([128, T], f32r, tag="attT")
                denB = [attpool.tile([1, T], f32r, tag=f"denB{hh}",
                                     name=f"denB{hh}")
                        for hh in range(HPC)]
                ats = {}
                stq = {}

                def emit_sc(qblk, j):
                    q0 = qblk * 512
                    c0 = max(0, 128 * j - q0)
                    diag = 128 * j >= q0
                    pt = ps.tile([128, 1024], f32, tag="sc",
                                 name=f"sc{qblk}_{j}")
                    for h in range(HPC):
                        qh = qT[64 * h:64 * h + 64, :]
                        kh = kT[64 * h:64 * h + 64, :]
                        o = 512 * h
                        nc.tensor.matmul(pt[:, o + c0:o + 512],
                                         kh[:, 128 * j:128 * j + 128],
                                         qh[:, q0 + c0:q0 + 512],
                                         start=True, stop=not diag)
                        if diag:
                            w = min(c0 + 128, 512)
                            nc.tensor.matmul(pt[:, o + c0:o + w], idr_sb[:],
                                             tri_sb[:, 0:w - c0],
                                             start=False, stop=True)
                    st_t = stpool.tile([128, 1024], f32r, tag="st",
                                       name=f"st{qblk}_{j}")
                    if c0 == 0:
                        nc.scalar.activation(st_t[:, 0:1024], pt[:, 0:1024],
                                             AF.Exp, scale=0.125)
                    else:
                        for h in range(HPC):
                            o = 512 * h
                            nc.scalar.activation(st_t[:, o + c0:o + 512],
                                                 pt[:, o + c0:o + 512],
                                                 AF.Exp, scale=0.125)
                    stq[(qblk, j)] = (st_t, c0)

                def emit_av(qblk, j):
                    njt = 4 * qblk + 4
                    st_t, c0 = stq.pop((qblk, j))
                    if j == 0:
                        ats[qblk] = [psat.tile([65, 512], f32, tag="at",
                                               name=f"at{qblk}_{hh}")
                                     for hh in range(HPC)]
                    at = ats[qblk]
                    for h in range(HPC):
                        o = 512 * h
                        nc.tensor.matmul(at[h][:, c0:512],
                                         v_b[:, j, 65 * h:65 * h + 65],
                                         st_t[:, o + c0:o + 512],
                                         start=(j == 0), stop=(j == njt - 1))
                    if j == njt - 1:
                        # evacuate unnormalized att + denominators
                        q0 = qblk * 512
                        for h in range(HPC):
                            nc.vector.tensor_copy(
                                attT[64 * h:64 * h + 64, q0:q0 + 512],
                                at[h][0:64, :])
                            nc.vector.tensor_copy(denB[h][0:1, q0:q0 + 512],
                                                  at[h][64:65, :])
                        del ats[qblk]

                seq = [(qblk, j) for qblk in range(4)
                       for j in range(4 * qblk + 4)]
                for i, (qblk, j) in enumerate(seq):
                    if parts != "p12se" and i >= LAG:
                        emit_av(*seq[i - LAG])
                    emit_sc(qblk, j)
                    if parts == "p12se":
                        stq.pop((qblk, j))
                    if i == 3 and mid_emit is not None:
                        mid_emit()
                if parts != "p12se":
                    for qblk, j in seq[len(seq) - LAG:]:
                        emit_av(qblk, j)
                return attT, denB

            def emit_ship(b, attT, denB):
                nc.sync.dma_start(
                    agin[b].ap()[:, 0:128, :].rearrange("j p t -> p j t"),
                    attT[:].rearrange("p (j t) -> p j t", t=TSL))
                for hh in range(HPC):
                    nc.sync.dma_start(
                        agin[b].ap()[:, 128 + hh:129 + hh, :].rearrange(
                            "j d t -> d j t"),
                        denB[hh][:].rearrange("d (j t) -> d j t", t=TSL))
                if with_collective:
                    nc.gpsimd.collective_compute(
                        "AllToAll", mybir.AluOpType.bypass,
                        replica_groups=[list(range(NCORES))],
                        ins=[agin[b].ap().opt()], outs=[agout[b].ap().opt()])

            def emit_outproj(b):
                """Output projection for this core's 256-token slice of batch
                b: un-normalize via selector matmul, then project."""
                src = (agout[b] if with_collective else agin[b]).ap()
                yh = yhpool.tile([128, NKT, TSL], f32r, tag="yh")
                nc.gpsimd.dma_start(
                    yh[:], src[:, 0:128, :].rearrange("k p t -> p k t"))
                den = smpool.tile([16, TSL], f32r, tag="den")
                for hh in range(HPC):
                    nc.gpsimd.dma_start(den[8 * hh:8 * hh + 8, :],
                                        src[:, 128 + hh, :])
                rec = smpool.tile([16, TSL], f32r, tag="rec")
                with nc.allow_low_precision(reason="bf16 recip feeds matmul"):
                    nc.vector.reciprocal(rec[:], den[:])
                yhs = yhpool.tile([128, NKT, TSL], f32r, tag="yhs")
                for kp in range(4):          # kk pairs: scale = sel^T @ rec
                    sc_ps = ps.tile([128, 512], f32, tag="sc",
                                    name=f"scale{b}_{kp}")
                    for u in range(2):
                        kk = 2 * kp + u
                        nc.tensor.matmul(sc_ps[:, u * TSL:(u + 1) * TSL],
                                         sel_sb[:, kk * 128:kk * 128 + 128],
                                         rec[:], start=True, stop=True)
                    nc.vector.tensor_mul(
                        yhs[:, 2 * kp:2 * kp + 2, :].rearrange("p a t -> p (a t)"),
                        yh[:, 2 * kp:2 * kp + 2, :].rearrange("p a t -> p (a t)"),
                        sc_ps[:])
                for tt in range(2):
                    y_sb = ypool.tile([128, D], f32, tag="y")
                    for eb in range(2):
                        pt = ps1.tile([128, 512], f32, tag="p1")
                        for kk in range(NKT):
                            nc.tensor.matmul(
                                pt[:],
                                yhs[:, kk, tt * 128:tt * 128 + 128],
                                wot_sb[:, kk, eb * 512:eb * 512 + 512],
                                start=(kk == 0), stop=False)
                        nc.tensor.matmul(pt[:], ones_sb[0:1, 0:128],
                                         bov_sb[0:1, eb * 512:eb * 512 + 512],
                                         start=False, stop=True)
                        nc.vector.tensor_copy(y_sb[:, eb * 512:eb * 512 + 512],
                                              pt[:])
                    nc.gpsimd.dma_start(
                        y.ap()[b * TSL + tt * 128: b * TSL + tt * 128 + 128, :],
                        y_sb[:])

            pending = []          # outproj batches shipped but not emitted
            xtiles = prefetch_x(0)
            for rep in range(repeat):
                for b in range(B):
                    qT, kT, v_b = emit_p1(b, xtiles)
                    if parts == "p1":
                        nc.sync.dma_start(agin[b].ap()[0, 0:128, :],
                                          qT[:, 0:TSL])
                        if not (rep == repeat - 1 and b == B - 1):
                            xtiles = prefetch_x((b + 1) % B)
                        continue
                    if not (rep == repeat - 1 and b == B - 1):
                        xtiles = prefetch_x((b + 1) % B)
                    mid = None
                    if len(pending) >= 2 or (pending and parts == "flush"):
                        pb = pending.pop(0)
                        mid = lambda pb=pb: emit_outproj(pb)
                    attT, denB = emit_p2(b, qT, kT, v_b, mid_emit=mid)
                    if parts == "p12se":
                        continue
                    emit_ship(b, attT, denB)
                    pending.append(b)
            while pending:
                emit_outproj(pending.pop(0))
    nc.compile()
    return nc


# ------------------------------------------------------------------
# Host-side wrapper
# ------------------------------------------------------------------
_CACHE = {}


def _prep_inputs(x, wq, bq, wk, bk, wv, bv, wo, bo, mdt="bf16"):
    if mdt == "bf16":
        import ml_dtypes
        cast = lambda a: np.asarray(a, ml_dtypes.bfloat16)
    else:
        cast = lambda a: np.asarray(a, np.float32)
    xt = np.ascontiguousarray(x.reshape(TB, D).T)          # [D, TB]

    def lhsT_pack(W):   # W [128, D] -> [128p, NKT, 128m]
        return np.ascontiguousarray(W.T.reshape(NKT, 128, 128).transpose(1, 0, 2))

    # triangular -1e30 mask constant (mask where col < p)
    cols = np.arange(128)
    trif = np.where(cols[None, :] < np.arange(128)[:, None], NEG, 0.0
                    ).astype(np.float32)

    # selector for receiver-side 1/Z broadcast: scale[p, kk-block col i] comes
    # from denom row 2*kk + (i >= 64)
    sel = np.zeros((16, NKT, 128), np.float32)
    for kk in range(NKT):
        sel[kk, kk, 0:64] = 1.0
        sel[8 + kk, kk, 64:128] = 1.0
    sel = sel.reshape(16, NKT * 128)

    wott = np.ascontiguousarray(wo.T.reshape(NKT, 128, D).transpose(1, 0, 2))
    bov = bo.reshape(1, D).astype(np.float32)
    idr = np.eye(128, dtype=np.float32)

    in_maps = []
    for c in range(NCORES):
        h0, h1 = HPC * c, HPC * c + 1
        Wq = np.concatenate([wq[h0], wq[h1]], axis=0)      # [128, D]
        Wk = np.concatenate([wk[h0], wk[h1]], axis=0)
        Wv = np.concatenate([wv[h0], wv[h1]], axis=0)
        wqkvp = np.concatenate([lhsT_pack(Wq), lhsT_pack(Wk), lhsT_pack(Wv)],
                               axis=2)                     # [128, NKT, 384]
        bqkvp = np.stack([np.concatenate([bq[h0], bq[h1]]),
                          np.concatenate([bk[h0], bk[h1]]),
                          np.concatenate([bv[h0], bv[h1]])], axis=1)  # [128,3]
        in_maps.append({
            "xt": cast(xt),
            "wqkv": cast(np.ascontiguousarray(wqkvp, np.float32)),
            "bqkv": np.ascontiguousarray(bqkvp, np.float32),
            "trif": cast(trif),
            "idr": cast(idr),
            "sel": cast(sel),
            "wot": cast(wott),
            "bov": cast(bov),
            "onesd": cast(np.ones((128, 16), np.float32)),
        })
    return in_maps


MDT = "bf16"   # matmul dtype: "bf16" or "f32r"


def kernel(x, wq, bq, wk, bk, wv, bv, wo, bo):
    from concourse import bass_utils
    x, wq, bq, wk, bk, wv, bv, wo, bo = (
        np.asarray(a, np.float32) for a in (x, wq, bq, wk, bk, wv, bv, wo, bo))
    if "nc" not in _CACHE:
        _CACHE["nc"] = build_nc(mdt=MDT)
    nc = _CACHE["nc"]
    in_maps = _prep_inputs(x, wq, bq, wk, bk, wv, bv, wo, bo, mdt=MDT)
    res = bass_utils.run_bass_kernel_spmd(nc, in_maps, core_ids=list(range(NCORES)))
    out = np.empty((B, T, D), np.float32)
    for c in range(NCORES):
        yc = res.results[c]["y"]        # [TS, D]: 4 batches x 256 tokens
        for b in range(B):
            out[b, TSL * c:TSL * (c + 1), :] = yc[TSL * b:TSL * (b + 1), :]
    return out


# revision 21
# speedup vs baseline: 2.3664x; 1.0403x over previous
"""Trainium2 Bass kernel for nn_MultiHeadAttention (B=4, T=2048, D=1024, H=16).

Sharding: tensor-parallel over heads - 2 heads per core on 8 cores.
Per core: QKV projections for its 2 heads, causal attention with scores kept
transposed (s^T[t_k, t_q]); the causal mask is a single 128-wide triangular
-1e30 constant accumulated into the diagonal score block on the tensor
engine; the softmax denominator comes from a ones-column appended to V.
Normalization is deferred: unnormalized attention + per-(head,token)
denominators ship through per-batch AllToAlls (head-split -> 256-token-slice
split), and the receiving core folds 1/Z in via a selector matmul before the
output projection. The output projection for batch b runs overlapped with
attention for batch b+1.

kernel(**inputs) takes the full unsharded inputs and returns the full output.
"""
import sys
sys.path.insert(0, "/opt/trn_rl_repo")
import numpy as np

B, T, D, H = 4, 2048, 1024, 16
DK = D // H           # 64
NCORES = 8
HPC = H // NCORES     # 2 heads per core
TB = B * T            # 8192 tokens
TS = TB // NCORES     # 1024 tokens of output per core (4 batches x 256)
NKT = D // 128        # 8 contraction k-tiles
TSL = T // NCORES     # 256-token A2A slice
NEG = -1.0e30
LAG = 3               # AV trails scores by LAG j-tiles


def build_nc(repeat=1, with_collective=True, parts="all", mdt="bf16"):
    import concourse.bacc as bacc
    import concourse.tile as tile
    import concourse.mybir as mybir

    f32 = mybir.dt.float32
    f32r = mybir.dt.bfloat16 if mdt == "bf16" else mybir.dt.float32r
    AF = mybir.ActivationFunctionType

    nc = bacc.Bacc("TRN2", target_bir_lowering=False, debug=False,
                   num_devices=NCORES)

    xt = nc.dram_tensor("xt", [D, TB], f32r, kind="ExternalInput")
    wqkv = nc.dram_tensor("wqkv", [128, NKT, 3 * 128], f32r, kind="ExternalInput")
    bqkv = nc.dram_tensor("bqkv", [128, 3], f32, kind="ExternalInput")
    trif = nc.dram_tensor("trif", [128, 128], f32r, kind="ExternalInput")
    idr = nc.dram_tensor("idr", [128, 128], f32r, kind="ExternalInput")
    self_sel = nc.dram_tensor("sel", [16, NKT * 128], f32r, kind="ExternalInput")
    wot = nc.dram_tensor("wot", [128, NKT, D], f32r, kind="ExternalInput")
    bov = nc.dram_tensor("bov", [1, D], f32r, kind="ExternalInput")
    onesd = nc.dram_tensor("onesd", [128, 16], f32r, kind="ExternalInput")
    y = nc.dram_tensor("y", [TS, D], f32, kind="ExternalOutput")

    # collective buffers: per-batch A2A payload = att rows 0:128 + denom 128:130
    agin = [nc.dram_tensor(f"agin{b}", [NCORES, 130, TSL], f32r)
            for b in range(B)]
    agout = [nc.dram_tensor(f"agout{b}", [NCORES, 130, TSL], f32r)
             for b in range(B)]

    xt_r = xt.ap().rearrange("(k p) t -> p k t", p=128)

    with tile.TileContext(nc) as tc:
        with tc.tile_pool(name="const", bufs=1) as cpool, \
             tc.tile_pool(name="xin", bufs=6) as xpool, \
             tc.tile_pool(name="qkv", bufs=2) as qkvpool, \
             tc.tile_pool(name="vt", bufs=4) as vtpool, \
             tc.tile_pool(name="st", bufs=4) as stpool, \
             tc.tile_pool(name="att", bufs=2) as attpool, \
             tc.tile_pool(name="yh", bufs=2) as yhpool, \
             tc.tile_pool(name="ysb", bufs=2) as ypool, \
             tc.tile_pool(name="small", bufs=4) as smpool, \
             tc.tile_pool(name="den", bufs=3) as denpool, \
             tc.tile_pool(name="ps", bufs=2, space="PSUM") as ps, \
             tc.tile_pool(name="ps1", bufs=2, space="PSUM") as ps1, \
             tc.tile_pool(name="psat", bufs=2, space="PSUM") as psat:

            # ---- constants ----
            wqkv_sb = cpool.tile([128, NKT, 3 * 128], f32r, tag="wqkv")
            nc.sync.dma_start(wqkv_sb[:], wqkv.ap())
            bqkv_sb = cpool.tile([128, 3], f32, tag="bqkv")
            nc.sync.dma_start(bqkv_sb[:], bqkv.ap())
            tri_sb = cpool.tile([128, 128], f32r, tag="tri")
            nc.sync.dma_start(tri_sb[:], trif.ap())
            idr_sb = cpool.tile([128, 128], f32r, tag="idr")
            nc.sync.dma_start(idr_sb[:], idr.ap())
            idn = cpool.tile([128, 128], f32 if mdt == "f32r" else f32r, tag="idn")
            nc.sync.dma_start(idn[:], idr.ap() if mdt == "bf16" else idr.ap().bitcast(f32))
            sel_sb = cpool.tile([16, NKT * 128], f32r, tag="sel")
            nc.sync.dma_start(sel_sb[:], self_sel.ap())
            wot_sb = cpool.tile([128, NKT, D], f32r, tag="wot")
            nc.sync.dma_start(wot_sb[:], wot.ap())
            bov_sb = cpool.tile([1, D], f32r, tag="bov")
            nc.sync.dma_start(bov_sb[:], bov.ap())
            ones16 = cpool.tile([128, 16], f32r, tag="ones16")
            nc.sync.dma_start(ones16[:], onesd.ap())
            ones_sb = cpool.tile([1, 128], f32r, tag="ones")
            nc.sync.dma_start(
                ones_sb[:],
                onesd.ap()[0:8, :].rearrange("p a -> (p a)").rearrange(
                    "(a b) -> a b", a=1))
            bovb = cpool.tile([128, D], f32r, tag="bovb")
            bc_ps = ps.tile([128, 1024], f32, tag="sc", name="bovbc")
            for half in range(2):
                nc.tensor.matmul(bc_ps[:, half * 512:half * 512 + 512],
                                 ones_sb[0:1, 0:128],
                                 bov_sb[0:1, half * 512:half * 512 + 512],
                                 start=True, stop=True)
            nc.vector.tensor_copy(bovb[:], bc_ps[:])

            def prefetch_x(b):
                tiles = []
                for blk in range(4):
                    t0 = b * T + blk * 512
                    xin = xpool.tile([128, NKT, 512], f32r, tag="xin",
                                     name=f"xin{b}_{blk}")
                    nc.sync.dma_start(xin[:], xt_r[:, :, t0:t0 + 512])
                    tiles.append(xin)
                return tiles

            def emit_p1(b, xtiles):
                """QKV projections + v transpose for batch b."""
                qT = qkvpool.tile([128, T], f32r, tag="qT")
                kT = qkvpool.tile([128, T], f32r, tag="kT")
                v_b = qkvpool.tile([128, 16, 130], f32r, tag="v")
                for blk in range(4):
                    xin = xtiles[blk]
                    vT = vtpool.tile([128, 512], f32 if mdt == "f32r" else f32r,
                                     tag="vT")
                    for pi, dest in ((0, qT), (1, kT), (2, vT)):
                        pt = ps1.tile([128, 512], f32, tag="p1")
                        for kk in range(NKT):
                            nc.tensor.matmul(pt[:],
                                             wqkv_sb[:, kk, pi * 128:pi * 128 + 128],
                                             xin[:, kk, :],
                                             start=(kk == 0), stop=(kk == NKT - 1))
                        if pi == 0:
                            nc.vector.tensor_scalar_add(
                                dest[:, blk * 512:blk * 512 + 512],
                                pt[:], bqkv_sb[:, 0:1])
                        elif pi == 1:
                            nc.vector.tensor_scalar_add(
                                dest[:, blk * 512:blk * 512 + 512],
                                pt[:], bqkv_sb[:, 1:2])
                        else:
                            nc.vector.tensor_scalar_add(dest[:, 0:512],
                                                        pt[:], bqkv_sb[:, 2:3])
                    # transpose v^T[128, 512] -> v tiles [t, 130]
                    pt = ps1.tile([128, 512], f32 if mdt == "f32r" else f32r,
                                  tag="p1")
                    for tt in range(4):
                        nc.tensor.transpose(pt[:, tt * 128:tt * 128 + 128],
                                            vT[:, tt * 128:tt * 128 + 128], idn[:])
                    for tt in range(4):
                        j = blk * 4 + tt
                        nc.vector.tensor_copy(v_b[:, j, 0:64],
                                              pt[:, tt * 128:tt * 128 + 64])
                        nc.vector.tensor_copy(v_b[:, j, 65:129],
                                              pt[:, tt * 128 + 64:tt * 128 + 128])
                nc.vector.tensor_copy(v_b[:, :, 64:65],
                                      ones16[:].rearrange("p (a c) -> p a c", c=1))
                nc.vector.tensor_copy(v_b[:, :, 129:130],
                                      ones16[:].rearrange("p (a c) -> p a c", c=1))
                return qT, kT, v_b

            def emit_p2(b, qT, kT, v_b, mid_emit=None):
                """Causal attention for batch b -> attT [128, T] + denB [2, T]
                (unnormalized; row 64 of each at tile is the denominator).
                One [128, 1024] score PSUM tile per j holds both heads, so
                exp is a single ACT instruction per j."""
                attT = attpool.tile([128, T], f32r, tag="attT")
                denB = [attpool.tile([1, T], f32r, tag=f"denB{hh}",
                                     name=f"denB{hh}")
                        for hh in range(HPC)]
                ats = {}
                stq = {}

                def emit_sc(qblk, j):
                    q0 = qblk * 512
                    c0 = max(0, 128 * j - q0)
                    diag = 128 * j >= q0
                    pt = ps.tile([128, 1024], f32, tag="sc",
                                 name=f"sc{qblk}_{j}")
                    for h in range(HPC):
                        qh = qT[64 * h:64 * h + 64, :]
                        kh = kT[64 * h:64 * h + 64, :]
                        o = 512 * h
                        nc.tensor.matmul(pt[:, o + c0:o + 512],
                                         kh[:, 128 * j:128 * j + 128],
                                         qh[:, q0 + c0:q0 + 512],
                                         start=True, stop=not diag)
                        if diag:
                            w = min(c0 + 128, 512)
                            nc.tensor.matmul(pt[:, o + c0:o + w], idr_sb[:],
                                             tri_sb[:, 0:w - c0],
                                             start=False, stop=True)
                    st_t = stpool.tile([128, 1024], f32r, tag="st",
                                       name=f"st{qblk}_{j}")
                    if c0 == 0:
                        nc.scalar.activation(st_t[:, 0:1024], pt[:, 0:1024],
                                             AF.Exp, scale=0.125)
                    else:
                        for h in range(HPC):
                            o = 512 * h
                            nc.scalar.activation(st_t[:, o + c0:o + 512],
                                                 pt[:, o + c0:o + 512],
                                                 AF.Exp, scale=0.125)
                    stq[(qblk, j)] = (st_t, c0)

                def emit_av(qblk, j):
                    njt = 4 * qblk + 4
                    st_t, c0 = stq.pop((qblk, j))
                    if j == 0:
                        ats[qblk] = [psat.tile([65, 512], f32, tag="at",
                                               name=f"at{qblk}_{hh}")
                                     for hh in range(HPC)]
                    at = ats[qblk]
                    for h in range(HPC):
                        o = 512 * h
                        nc.tensor.matmul(at[h][:, c0:512],
                                         v_b[:, j, 65 * h:65 * h + 65],
                                         st_t[:, o + c0:o + 512],
                                         start=(j == 0), stop=(j == njt - 1))
                    if j == njt - 1:
                        # evacuate unnormalized att + denominators
                        q0 = qblk * 512
                        for h in range(HPC):
                            nc.vector.tensor_copy(
                                attT[64 * h:64 * h + 64, q0:q0 + 512],
                                at[h][0:64, :])
                            nc.vector.tensor_copy(denB[h][0:1, q0:q0 + 512],
                                                  at[h][64:65, :])
                        del ats[qblk]

                seq = [(qblk, j) for qblk in range(4)
                       for j in range(4 * qblk + 4)]
                for i, (qblk, j) in enumerate(seq):
                    if parts != "p12se" and i >= LAG:
                        emit_av(*seq[i - LAG])
                    emit_sc(qblk, j)
                    if parts == "p12se":
                        stq.pop((qblk, j))
                    if i == 3 and mid_emit is not None:
                        mid_emit()
                if parts != "p12se":
                    for qblk, j in seq[len(seq) - LAG:]:
                        emit_av(qblk, j)
                return attT, denB

            def emit_ship(b, attT, denB):
                nc.sync.dma_start(
                    agin[b].ap()[:, 0:128, :].rearrange("j p t -> p j t"),
                    attT[:].rearrange("p (j t) -> p j t", t=TSL))
                for hh in range(HPC):
                    nc.sync.dma_start(
                        agin[b].ap()[:, 128 + hh:129 + hh, :].rearrange(
                            "j d t -> d j t"),
                        denB[hh][:].rearrange("d (j t) -> d j t", t=TSL))
                if with_collective:
                    nc.gpsimd.collective_compute(
                        "AllToAll", mybir.AluOpType.bypass,
                        replica_groups=[list(range(NCORES))],
                        ins=[agin[b].ap().opt()], outs=[agout[b].ap().opt()])

            heads = {}

            def emit_outproj_head(b):
                src = (agout[b] if with_collective else agin[b]).ap()
                yh = yhpool.tile([128, NKT, TSL], f32r, tag="yh")
                nc.gpsimd.dma_start(
                    yh[:], src[:, 0:128, :].rearrange("k p t -> p k t"))
                den = denpool.tile([16, TSL], f32r, tag="den")
                for hh in range(HPC):
                    nc.gpsimd.dma_start(den[8 * hh:8 * hh + 8, :],
                                        src[:, 128 + hh, :])
                heads[b] = (yh, den)

            def emit_outproj(b):
                """Output projection for this core's 256-token slice of batch
                b: un-normalize via selector matmul, then project."""
                yh, den = heads.pop(b)
                rec = smpool.tile([16, TSL], f32r, tag="rec")
                with nc.allow_low_precision(reason="bf16 recip feeds matmul"):
                    nc.vector.reciprocal(rec[:], den[:])
                yhs = yhpool.tile([128, NKT, TSL], f32r, tag="yhs")
                for kp in range(4):          # kk pairs: scale = sel^T @ rec
                    sc_ps = ps1.tile([128, 512], f32, tag="p1",
                                     name=f"scale{b}_{kp}")
                    for u in range(2):
                        kk = 2 * kp + u
                        nc.tensor.matmul(sc_ps[:, u * TSL:(u + 1) * TSL],
                                         sel_sb[:, kk * 128:kk * 128 + 128],
                                         rec[:], start=True, stop=True)
                    nc.vector.tensor_mul(
                        yhs[:, 2 * kp:2 * kp + 2, :].rearrange("p a t -> p (a t)"),
                        yh[:, 2 * kp:2 * kp + 2, :].rearrange("p a t -> p (a t)"),
                        sc_ps[:])
                for tt in range(2):
                    y_sb = ypool.tile([128, D], f32, tag="y")
                    for eb in range(2):
                        pt = ps1.tile([128, 512], f32, tag="p1")
                        for kk in range(NKT):
                            nc.tensor.matmul(
                                pt[:],
                                yhs[:, kk, tt * 128:tt * 128 + 128],
                                wot_sb[:, kk, eb * 512:eb * 512 + 512],
                                start=(kk == 0), stop=(kk == NKT - 1))
                        nc.vector.tensor_tensor(
                            y_sb[:, eb * 512:eb * 512 + 512], pt[:],
                            bovb[:, eb * 512:eb * 512 + 512],
                            mybir.AluOpType.add)
                    nc.gpsimd.dma_start(
                        y.ap()[b * TSL + tt * 128: b * TSL + tt * 128 + 128, :],
                        y_sb[:])

            pending = []          # outproj batches shipped but not emitted
            xtiles = prefetch_x(0)
            for rep in range(repeat):
                for b in range(B):
                    xt_cur = xtiles
                    if not (rep == repeat - 1 and b == B - 1):
                        xtiles = prefetch_x((b + 1) % B)
                    qT, kT, v_b = emit_p1(b, xt_cur)
                    if parts == "p1":
                        nc.sync.dma_start(agin[b].ap()[0, 0:128, :],
                                          qT[:, 0:TSL])
                        continue
                    mid = None
                    if len(pending) >= 2 or (pending and parts == "flush"):
                        pb = pending.pop(0)
                        mid = lambda pb=pb: emit_outproj(pb)
                    attT, denB = emit_p2(b, qT, kT, v_b, mid_emit=mid)
                    if parts == "p12se":
                        continue
                    emit_ship(b, attT, denB)
                    emit_outproj_head(b)
                    pending.append(b)
            while pending:
                emit_outproj(pending.pop(0))
    nc.compile()
    return nc


# ------------------------------------------------------------------
# Host-side wrapper
# ------------------------------------------------------------------
_CACHE = {}


def _prep_inputs(x, wq, bq, wk, bk, wv, bv, wo, bo, mdt="bf16"):
    if mdt == "bf16":
        import ml_dtypes
        cast = lambda a: np.asarray(a, ml_dtypes.bfloat16)
    else:
        cast = lambda a: np.asarray(a, np.float32)
    xt = np.ascontiguousarray(x.reshape(TB, D).T)          # [D, TB]

    def lhsT_pack(W):   # W [128, D] -> [128p, NKT, 128m]
        return np.ascontiguousarray(W.T.reshape(NKT, 128, 128).transpose(1, 0, 2))

    # triangular -1e30 mask constant (mask where col < p)
    cols = np.arange(128)
    trif = np.where(cols[None, :] < np.arange(128)[:, None], NEG, 0.0
                    ).astype(np.float32)

    # selector for receiver-side 1/Z broadcast: scale[p, kk-block col i] comes
    # from denom row 2*kk + (i >= 64)
    sel = np.zeros((16, NKT, 128), np.float32)
    for kk in range(NKT):
        sel[kk, kk, 0:64] = 1.0
        sel[8 + kk, kk, 64:128] = 1.0
    sel = sel.reshape(16, NKT * 128)

    wott = np.ascontiguousarray(wo.T.reshape(NKT, 128, D).transpose(1, 0, 2))
    bov = bo.reshape(1, D).astype(np.float32)
    idr = np.eye(128, dtype=np.float32)

    in_maps = []
    for c in range(NCORES):
        h0, h1 = HPC * c, HPC * c + 1
        Wq = np.concatenate([wq[h0], wq[h1]], axis=0)      # [128, D]
        Wk = np.concatenate([wk[h0], wk[h1]], axis=0)
        Wv = np.concatenate([wv[h0], wv[h1]], axis=0)
        wqkvp = np.concatenate([lhsT_pack(Wq), lhsT_pack(Wk), lhsT_pack(Wv)],
                               axis=2)                     # [128, NKT, 384]
        bqkvp = np.stack([np.concatenate([bq[h0], bq[h1]]),
                          np.concatenate([bk[h0], bk[h1]]),
                          np.concatenate([bv[h0], bv[h1]])], axis=1)  # [128,3]
        in_maps.append({
            "xt": cast(xt),
            "wqkv": cast(np.ascontiguousarray(wqkvp, np.float32)),
            "bqkv": np.ascontiguousarray(bqkvp, np.float32),
            "trif": cast(trif),
            "idr": cast(idr),
            "sel": cast(sel),
            "wot": cast(wott),
            "bov": cast(bov),
            "onesd": cast(np.ones((128, 16), np.float32)),
        })
    return in_maps


MDT = "bf16"   # matmul dtype: "bf16" or "f32r"


def kernel(x, wq, bq, wk, bk, wv, bv, wo, bo):
    from concourse import bass_utils
    x, wq, bq, wk, bk, wv, bv, wo, bo = (
        np.asarray(a, np.float32) for a in (x, wq, bq, wk, bk, wv, bv, wo, bo))
    if "nc" not in _CACHE:
        _CACHE["nc"] = build_nc(mdt=MDT)
    nc = _CACHE["nc"]
    in_maps = _prep_inputs(x, wq, bq, wk, bk, wv, bv, wo, bo, mdt=MDT)
    res = bass_utils.run_bass_kernel_spmd(nc, in_maps, core_ids=list(range(NCORES)))
    out = np.empty((B, T, D), np.float32)
    for c in range(NCORES):
        yc = res.results[c]["y"]        # [TS, D]: 4 batches x 256 tokens
        for b in range(B):
            out[b, TSL * c:TSL * (c + 1), :] = yc[TSL * b:TSL * (b + 1), :]
    return out


# revision 25
# speedup vs baseline: 3.7340x; 1.5780x over previous
"""Trainium2 Bass kernel for nn_MultiHeadAttention (B=4, T=2048, D=1024, H=16).

Sharding: tensor-parallel over heads - 2 heads per core on 8 cores.
Per core: QKV projections for its 2 heads, causal attention with scores kept
transposed (s^T[t_k, t_q]); the causal mask is a single 128-wide triangular
-1e30 constant accumulated into the diagonal score block on the tensor
engine; the softmax denominator comes from a ones-column appended to V.
Normalization is deferred: unnormalized attention + per-(head,token)
denominators ship through per-batch AllToAlls (head-split -> 256-token-slice
split), and the receiving core folds 1/Z in via a selector matmul before the
output projection. The output projection for batch b runs overlapped with
attention for batch b+1.

kernel(**inputs) takes the full unsharded inputs and returns the full output.
"""
import sys
sys.path.insert(0, "/opt/trn_rl_repo")
import numpy as np

B, T, D, H = 4, 2048, 1024, 16
DK = D // H           # 64
NCORES = 8
HPC = H // NCORES     # 2 heads per core
TB = B * T            # 8192 tokens
TS = TB // NCORES     # 1024 tokens of output per core (4 batches x 256)
NKT = D // 128        # 8 contraction k-tiles
TSL = T // NCORES     # 256-token A2A slice
NEG = -1.0e30
LAG = 3               # AV trails scores by LAG j-tiles


def build_nc(repeat=1, with_collective=True, parts="all", mdt="bf16"):
    import concourse.bacc as bacc
    import concourse.tile as tile
    import concourse.mybir as mybir

    f32 = mybir.dt.float32
    f32r = mybir.dt.bfloat16 if mdt == "bf16" else mybir.dt.float32r
    AF = mybir.ActivationFunctionType

    nc = bacc.Bacc("TRN2", target_bir_lowering=False, debug=False,
                   num_devices=NCORES)

    xt = nc.dram_tensor("xt", [D, TB], f32r, kind="ExternalInput")
    wqkv = nc.dram_tensor("wqkv", [128, NKT, 3 * 128], f32r, kind="ExternalInput")
    bqkv = nc.dram_tensor("bqkv", [128, 3], f32, kind="ExternalInput")
    trif = nc.dram_tensor("trif", [128, 128], f32r, kind="ExternalInput")
    idr = nc.dram_tensor("idr", [128, 128], f32r, kind="ExternalInput")
    self_sel = nc.dram_tensor("sel", [16, NKT * 128], f32r, kind="ExternalInput")
    wot = nc.dram_tensor("wot", [128, NKT, D], f32r, kind="ExternalInput")
    bov = nc.dram_tensor("bov", [1, D], f32r, kind="ExternalInput")
    onesd = nc.dram_tensor("onesd", [128, 16], f32r, kind="ExternalInput")
    y = nc.dram_tensor("y", [TS, D], f32, kind="ExternalOutput")

    # collective buffers: per-batch A2A payload = att rows 0:128 + denom 128:130
    agin = [nc.dram_tensor(f"agin{b}", [NCORES, 130, TSL], f32r)
            for b in range(B)]
    agout = [nc.dram_tensor(f"agout{b}", [NCORES, 130, TSL], f32r)
             for b in range(B)]

    xt_r = xt.ap().rearrange("(k p) t -> p k t", p=128)

    with tile.TileContext(nc) as tc:
        with tc.tile_pool(name="const", bufs=1) as cpool, \
             tc.tile_pool(name="xin", bufs=6) as xpool, \
             tc.tile_pool(name="qkv", bufs=2) as qkvpool, \
             tc.tile_pool(name="vt", bufs=4) as vtpool, \
             tc.tile_pool(name="st", bufs=4) as stpool, \
             tc.tile_pool(name="att", bufs=2) as attpool, \
             tc.tile_pool(name="yh", bufs=2) as yhpool, \
             tc.tile_pool(name="ysb", bufs=2) as ypool, \
             tc.tile_pool(name="small", bufs=4) as smpool, \
             tc.tile_pool(name="den", bufs=3) as denpool, \
             tc.tile_pool(name="ps", bufs=2, space="PSUM") as ps, \
             tc.tile_pool(name="ps1", bufs=2, space="PSUM") as ps1, \
             tc.tile_pool(name="psat", bufs=2, space="PSUM") as psat:

            # ---- constants (wqkv first: it gates the first matmul; the
            # rest are needed progressively later) ----
            wqkv_sb = cpool.tile([128, NKT, 3 * 128], f32r, tag="wqkv")
            nc.sync.dma_start(wqkv_sb[:], wqkv.ap())
            bqkv_sb = cpool.tile([128, 3], f32, tag="bqkv")
            nc.sync.dma_start(bqkv_sb[:], bqkv.ap())
            idr_sb = cpool.tile([128, 128], f32r, tag="idr")
            nc.sync.dma_start(idr_sb[:], idr.ap())
            idn = cpool.tile([128, 128], f32 if mdt == "f32r" else f32r, tag="idn")
            nc.sync.dma_start(idn[:], idr.ap() if mdt == "bf16" else idr.ap().bitcast(f32))
            ones16 = cpool.tile([128, 16], f32r, tag="ones16")
            nc.sync.dma_start(ones16[:], onesd.ap())
            tri_sb = cpool.tile([128, 128], f32r, tag="tri")
            sel_sb = cpool.tile([16, NKT * 128], f32r, tag="sel")
            wot_sb = cpool.tile([128, NKT, D], f32r, tag="wot")
            bov_sb = cpool.tile([1, D], f32r, tag="bov")

            def emit_late_consts():
                nc.sync.dma_start(tri_sb[:], trif.ap())
                nc.sync.dma_start(sel_sb[:], self_sel.ap())
                nc.sync.dma_start(wot_sb[:], wot.ap())
                nc.sync.dma_start(bov_sb[:], bov.ap())
            ones_sb = cpool.tile([1, 128], f32r, tag="ones")
            nc.sync.dma_start(
                ones_sb[:],
                onesd.ap()[0:8, :].rearrange("p a -> (p a)").rearrange(
                    "(a b) -> a b", a=1))
            bovb = cpool.tile([128, D], f32r, tag="bovb")

            def emit_bovb():
                bc_ps = ps.tile([128, 1024], f32, tag="sc", name="bovbc")
                for half in range(2):
                    nc.tensor.matmul(bc_ps[:, half * 512:half * 512 + 512],
                                     ones_sb[0:1, 0:128],
                                     bov_sb[0:1, half * 512:half * 512 + 512],
                                     start=True, stop=True)
                nc.vector.tensor_copy(bovb[:], bc_ps[:])

            def prefetch_x(b):
                tiles = []
                for blk in range(4):
                    t0 = b * T + blk * 512
                    xin = xpool.tile([128, NKT, 512], f32r, tag="xin",
                                     name=f"xin{b}_{blk}")
                    nc.sync.dma_start(xin[:], xt_r[:, :, t0:t0 + 512])
                    tiles.append(xin)
                return tiles

            def emit_p1(b, xtiles):
                """QKV projections + v transpose for batch b."""
                qT = qkvpool.tile([128, T], f32r, tag="qT")
                kT = qkvpool.tile([128, T], f32r, tag="kT")
                v_b = qkvpool.tile([128, 16, 130], f32r, tag="v")
                for blk in range(4):
                    xin = xtiles[blk]
                    vT = vtpool.tile([128, 512], f32 if mdt == "f32r" else f32r,
                                     tag="vT")
                    for pi, dest in ((0, qT), (1, kT), (2, vT)):
                        pt = ps1.tile([128, 512], f32, tag="p1")
                        for kk in range(NKT):
                            nc.tensor.matmul(pt[:],
                                             wqkv_sb[:, kk, pi * 128:pi * 128 + 128],
                                             xin[:, kk, :],
                                             start=(kk == 0), stop=(kk == NKT - 1))
                        if pi == 0:
                            nc.vector.tensor_scalar_add(
                                dest[:, blk * 512:blk * 512 + 512],
                                pt[:], bqkv_sb[:, 0:1])
                        elif pi == 1:
                            nc.vector.tensor_scalar_add(
                                dest[:, blk * 512:blk * 512 + 512],
                                pt[:], bqkv_sb[:, 1:2])
                        else:
                            nc.vector.tensor_scalar_add(dest[:, 0:512],
                                                        pt[:], bqkv_sb[:, 2:3])
                    # transpose v^T[128, 512] -> v tiles [t, 130]
                    pt = ps1.tile([128, 512], f32 if mdt == "f32r" else f32r,
                                  tag="p1")
                    for tt in range(4):
                        nc.tensor.transpose(pt[:, tt * 128:tt * 128 + 128],
                                            vT[:, tt * 128:tt * 128 + 128], idn[:])
                    for tt in range(4):
                        j = blk * 4 + tt
                        nc.vector.tensor_copy(v_b[:, j, 0:64],
                                              pt[:, tt * 128:tt * 128 + 64])
                        nc.vector.tensor_copy(v_b[:, j, 65:129],
                                              pt[:, tt * 128 + 64:tt * 128 + 128])
                nc.vector.tensor_copy(v_b[:, :, 64:65],
                                      ones16[:].rearrange("p (a c) -> p a c", c=1))
                nc.vector.tensor_copy(v_b[:, :, 129:130],
                                      ones16[:].rearrange("p (a c) -> p a c", c=1))
                return qT, kT, v_b

            def emit_p2(b, qT, kT, v_b, mid_emit=None):
                """Causal attention for batch b -> attT [128, T] + denB [2, T]
                (unnormalized; row 64 of each at tile is the denominator).
                One [128, 1024] score PSUM tile per j holds both heads, so
                exp is a single ACT instruction per j."""
                attT = attpool.tile([128, T], f32r, tag="attT")
                denB = [attpool.tile([1, T], f32r, tag=f"denB{hh}",
                                     name=f"denB{hh}")
                        for hh in range(HPC)]
                ats = {}
                stq = {}

                def emit_sc(qblk, j):
                    q0 = qblk * 512
                    c0 = max(0, 128 * j - q0)
                    diag = 128 * j >= q0
                    pt = ps.tile([128, 1024], f32, tag="sc",
                                 name=f"sc{qblk}_{j}")
                    for h in range(HPC):
                        qh = qT[64 * h:64 * h + 64, :]
                        kh = kT[64 * h:64 * h + 64, :]
                        o = 512 * h
                        nc.tensor.matmul(pt[:, o + c0:o + 512],
                                         kh[:, 128 * j:128 * j + 128],
                                         qh[:, q0 + c0:q0 + 512],
                                         start=True, stop=not diag)
                        if diag:
                            w = min(c0 + 128, 512)
                            nc.tensor.matmul(pt[:, o + c0:o + w], idr_sb[:],
                                             tri_sb[:, 0:w - c0],
                                             start=False, stop=True)
                    st_t = stpool.tile([128, 1024], f32r, tag="st",
                                       name=f"st{qblk}_{j}")
                    if c0 == 0:
                        nc.scalar.activation(st_t[:, 0:1024], pt[:, 0:1024],
                                             AF.Exp, scale=0.125)
                    else:
                        for h in range(HPC):
                            o = 512 * h
                            nc.scalar.activation(st_t[:, o + c0:o + 512],
                                                 pt[:, o + c0:o + 512],
                                                 AF.Exp, scale=0.125)
                    stq[(qblk, j)] = (st_t, c0)

                def emit_av(qblk, j):
                    njt = 4 * qblk + 4
                    st_t, c0 = stq.pop((qblk, j))
                    if j == 0:
                        ats[qblk] = [psat.tile([65, 512], f32, tag="at",
                                               name=f"at{qblk}_{hh}")
                                     for hh in range(HPC)]
                    at = ats[qblk]
                    for h in range(HPC):
                        o = 512 * h
                        nc.tensor.matmul(at[h][:, c0:512],
                                         v_b[:, j, 65 * h:65 * h + 65],
                                         st_t[:, o + c0:o + 512],
                                         start=(j == 0), stop=(j == njt - 1))
                    if j == njt - 1:
                        # evacuate unnormalized att + denominators
                        q0 = qblk * 512
                        for h in range(HPC):
                            nc.vector.tensor_copy(
                                attT[64 * h:64 * h + 64, q0:q0 + 512],
                                at[h][0:64, :])
                            nc.vector.tensor_copy(denB[h][0:1, q0:q0 + 512],
                                                  at[h][64:65, :])
                        del ats[qblk]

                seq = [(qblk, j) for qblk in range(4)
                       for j in range(4 * qblk + 4)]
                for i, (qblk, j) in enumerate(seq):
                    if parts != "p12se" and i >= LAG:
                        emit_av(*seq[i - LAG])
                    emit_sc(qblk, j)
                    if parts == "p12se":
                        stq.pop((qblk, j))
                    if i == 3 and mid_emit is not None:
                        mid_emit()
                if parts != "p12se":
                    for qblk, j in seq[len(seq) - LAG:]:
                        emit_av(qblk, j)
                return attT, denB

            def emit_ship(b, attT, denB):
                nc.sync.dma_start(
                    agin[b].ap()[:, 0:128, :].rearrange("j p t -> p j t"),
                    attT[:].rearrange("p (j t) -> p j t", t=TSL))
                for hh in range(HPC):
                    nc.sync.dma_start(
                        agin[b].ap()[:, 128 + hh:129 + hh, :].rearrange(
                            "j d t -> d j t"),
                        denB[hh][:].rearrange("d (j t) -> d j t", t=TSL))
                if with_collective:
                    nc.gpsimd.collective_compute(
                        "AllToAll", mybir.AluOpType.bypass,
                        replica_groups=[list(range(NCORES))],
                        ins=[agin[b].ap().opt()], outs=[agout[b].ap().opt()])

            heads = {}

            def emit_outproj_head(b):
                src = (agout[b] if with_collective else agin[b]).ap()
                yh = yhpool.tile([128, NKT, TSL], f32r, tag="yh")
                nc.gpsimd.dma_start(
                    yh[:], src[:, 0:128, :].rearrange("k p t -> p k t"))
                den = denpool.tile([16, TSL], f32r, tag="den")
                for hh in range(HPC):
                    nc.gpsimd.dma_start(den[8 * hh:8 * hh + 8, :],
                                        src[:, 128 + hh, :])
                heads[b] = (yh, den)

            def emit_outproj(b):
                """Output projection for this core's 256-token slice of batch
                b: un-normalize via selector matmul, then project."""
                yh, den = heads.pop(b)
                rec = smpool.tile([16, TSL], f32r, tag="rec")
                with nc.allow_low_precision(reason="bf16 recip feeds matmul"):
                    nc.vector.reciprocal(rec[:], den[:])
                yhs = yhpool.tile([128, NKT, TSL], f32r, tag="yhs")
                for kp in range(4):          # kk pairs: scale = sel^T @ rec
                    sc_ps = ps1.tile([128, 512], f32, tag="p1",
                                     name=f"scale{b}_{kp}")
                    for u in range(2):
                        kk = 2 * kp + u
                        nc.tensor.matmul(sc_ps[:, u * TSL:(u + 1) * TSL],
                                         sel_sb[:, kk * 128:kk * 128 + 128],
                                         rec[:], start=True, stop=True)
                    nc.vector.tensor_mul(
                        yhs[:, 2 * kp:2 * kp + 2, :].rearrange("p a t -> p (a t)"),
                        yh[:, 2 * kp:2 * kp + 2, :].rearrange("p a t -> p (a t)"),
                        sc_ps[:])
                for tt in range(2):
                    y_sb = ypool.tile([128, D], f32, tag="y")
                    for eb in range(2):
                        pt = ps1.tile([128, 512], f32, tag="p1")
                        for kk in range(NKT):
                            nc.tensor.matmul(
                                pt[:],
                                yhs[:, kk, tt * 128:tt * 128 + 128],
                                wot_sb[:, kk, eb * 512:eb * 512 + 512],
                                start=(kk == 0), stop=(kk == NKT - 1))
                        nc.vector.tensor_tensor(
                            y_sb[:, eb * 512:eb * 512 + 512], pt[:],
                            bovb[:, eb * 512:eb * 512 + 512],
                            mybir.AluOpType.add)
                    nc.gpsimd.dma_start(
                        y.ap()[b * TSL + tt * 128: b * TSL + tt * 128 + 128, :],
                        y_sb[:])

            pending = []          # outproj batches shipped but not emitted
            xtiles = prefetch_x(0)
            emit_late_consts()
            emit_bovb()
            for rep in range(repeat):
                for b in range(B):
                    xt_cur = xtiles
                    if not (rep == repeat - 1 and b == B - 1):
                        xtiles = prefetch_x((b + 1) % B)
                    qT, kT, v_b = emit_p1(b, xt_cur)
                    if parts == "p1":
                        nc.sync.dma_start(agin[b].ap()[0, 0:128, :],
                                          qT[:, 0:TSL])
                        continue
                    mid = None
                    if len(pending) >= 2 or (pending and parts == "flush"):
                        pb = pending.pop(0)
                        mid = lambda pb=pb: emit_outproj(pb)
                    attT, denB = emit_p2(b, qT, kT, v_b, mid_emit=mid)
                    if parts == "p12se":
                        continue
                    emit_ship(b, attT, denB)
                    emit_outproj_head(b)
                    pending.append(b)
            while pending:
                emit_outproj(pending.pop(0))
    nc.compile()
    return nc


# ------------------------------------------------------------------
# Host-side wrapper
# ------------------------------------------------------------------
_CACHE = {}


def _prep_inputs(x, wq, bq, wk, bk, wv, bv, wo, bo, mdt="bf16"):
    if mdt == "bf16":
        import ml_dtypes
        cast = lambda a: np.asarray(a, ml_dtypes.bfloat16)
    else:
        cast = lambda a: np.asarray(a, np.float32)
    xt = np.ascontiguousarray(x.reshape(TB, D).T)          # [D, TB]

    def lhsT_pack(W):   # W [128, D] -> [128p, NKT, 128m]
        return np.ascontiguousarray(W.T.reshape(NKT, 128, 128).transpose(1, 0, 2))

    # triangular -1e30 mask constant (mask where col < p)
    cols = np.arange(128)
    trif = np.where(cols[None, :] < np.arange(128)[:, None], NEG, 0.0
                    ).astype(np.float32)

    # selector for receiver-side 1/Z broadcast: scale[p, kk-block col i] comes
    # from denom row 2*kk + (i >= 64)
    sel = np.zeros((16, NKT, 128), np.float32)
    for kk in range(NKT):
        sel[kk, kk, 0:64] = 1.0
        sel[8 + kk, kk, 64:128] = 1.0
    sel = sel.reshape(16, NKT * 128)

    wott = np.ascontiguousarray(wo.T.reshape(NKT, 128, D).transpose(1, 0, 2))
    bov = bo.reshape(1, D).astype(np.float32)
    idr = np.eye(128, dtype=np.float32)

    in_maps = []
    for c in range(NCORES):
        h0, h1 = HPC * c, HPC * c + 1
        Wq = np.concatenate([wq[h0], wq[h1]], axis=0)      # [128, D]
        Wk = np.concatenate([wk[h0], wk[h1]], axis=0)
        Wv = np.concatenate([wv[h0], wv[h1]], axis=0)
        wqkvp = np.concatenate([lhsT_pack(Wq), lhsT_pack(Wk), lhsT_pack(Wv)],
                               axis=2)                     # [128, NKT, 384]
        bqkvp = np.stack([np.concatenate([bq[h0], bq[h1]]),
                          np.concatenate([bk[h0], bk[h1]]),
                          np.concatenate([bv[h0], bv[h1]])], axis=1)  # [128,3]
        in_maps.append({
            "xt": cast(xt),
            "wqkv": cast(np.ascontiguousarray(wqkvp, np.float32)),
            "bqkv": np.ascontiguousarray(bqkvp, np.float32),
            "trif": cast(trif),
            "idr": cast(idr),
            "sel": cast(sel),
            "wot": cast(wott),
            "bov": cast(bov),
            "onesd": cast(np.ones((128, 16), np.float32)),
        })
    return in_maps


MDT = "bf16"   # matmul dtype: "bf16" or "f32r"


def kernel(x, wq, bq, wk, bk, wv, bv, wo, bo):
    from concourse import bass_utils
    x, wq, bq, wk, bk, wv, bv, wo, bo = (
        np.asarray(a, np.float32) for a in (x, wq, bq, wk, bk, wv, bv, wo, bo))
    if "nc" not in _CACHE:
        _CACHE["nc"] = build_nc(mdt=MDT)
    nc = _CACHE["nc"]
    in_maps = _prep_inputs(x, wq, bq, wk, bk, wv, bv, wo, bo, mdt=MDT)
    res = bass_utils.run_bass_kernel_spmd(nc, in_maps, core_ids=list(range(NCORES)))
    out = np.empty((B, T, D), np.float32)
    for c in range(NCORES):
        yc = res.results[c]["y"]        # [TS, D]: 4 batches x 256 tokens
        for b in range(B):
            out[b, TSL * c:TSL * (c + 1), :] = yc[TSL * b:TSL * (b + 1), :]
    return out
